# revision 1
# baseline (speedup 1.0000x reference)
"""GQA attention kernel for 8 Trainium2 NeuronCores.

Sharding: core c handles batch b = c//4, query rows [512*(c%4), 512*(c%4)+512).
Each core computes K/V for its batch's full sequence (4x replication of the tiny
kv projection), all 16 heads of attention for its 512 query rows, and the final
projection for its output chunk. No collectives needed.

Device layouts are all "transposed" (contraction/head dim on partitions):
  xfT [E,N] (rolled so this core's q rows are cols 0:512), qT/kT [m, r|n],
  v [n, dv] with a ones column appended per kv head (gives the softmax
  denominator for free in the attn@v matmul).
RMSNorm across partitions uses mask-matmuls; RoPE's half-rotation uses a
permutation matmul; per-column broadcasts use rank-1 matmuls.
Q heads are reordered on the host (wq cols / bq / proj rows) so each q head's
partition row (0/64) matches its kv head's row, satisfying the PE constraint
that lhsT and rhs share a base partition.
"""

import numpy as np

import concourse.bass as bass
import concourse.tile as tile
from concourse import bacc, mybir
from concourse import bass_utils

B, N, E = 2, 2048, 1024
H, KV, D = 16, 4, 64
R = 512            # query rows per core
EPS = 1e-6
F32 = mybir.dt.float32
BF16 = mybir.dt.bfloat16
AF = mybir.ActivationFunctionType

# head order: tile t holds (HEAD_ORDER[2t] at rows 0:64, HEAD_ORDER[2t+1] at 64:128)
HEAD_ORDER = [0, 4, 1, 5, 2, 6, 3, 7, 8, 12, 9, 13, 10, 14, 11, 15]

# dtype knobs
QKV_MM_DT = BF16    # dtype of x/w operands for q/k/v projections
ATT_DT = BF16      # dtype of qn/kn for the scores matmul
ES_DT = BF16       # dtype of exp(scores) and v for the attn@v matmul
PROJ_DT = BF16      # dtype of attn-out/proj_w operands for final projection


def _norm_rope_block(nc, s1t, s1p, raw_sl, out_sl, cos_sl, sin_sl,
                     smk_t, bcm_t, p2_t, eps_t):
    """raw [128,512] (2 heads) -> rmsnorm+rope -> out_sl (may be bf16)."""
    sqk = s1t.tile([128, 512], F32, tag="sqk", bufs=2, name="sqk")
    nc.vector.tensor_mul(sqk, raw_sl, raw_sl)
    pks = s1p.tile([2, 512], F32, tag="s1pp", bufs=2, name="pks")
    nc.tensor.matmul(pks, smk_t, sqk, start=True, stop=True)
    sdk = s1t.tile([2, 512], F32, tag="sdk", bufs=2, name="sdk")
    nc.scalar.activation(out=sdk, in_=pks, func=AF.Sqrt,
                         bias=eps_t[0:2], scale=1.0 / 64)
    rsv = s1t.tile([2, 512], F32, tag="rsv", bufs=2, name="rsv")
    nc.vector.reciprocal(out=rsv, in_=sdk)
    psw = s1p.tile([128, 512], F32, tag="s1sw", bufs=2, name="psw")
    nc.tensor.matmul(psw, p2_t, raw_sl, start=True, stop=True)
    prb = s1p.tile([128, 512], F32, tag="s1rb", bufs=2, name="prb")
    nc.tensor.matmul(prb, bcm_t, rsv, start=True, stop=True)
    t1 = s1t.tile([128, 512], F32, tag="t1", name="t1")
    nc.vector.tensor_mul(t1, raw_sl, cos_sl)
    t2 = s1t.tile([128, 512], F32, tag="t2", name="t2")
    nc.vector.tensor_mul(t2, psw, sin_sl)
    nc.vector.tensor_add(t1, t1, t2)
    nc.vector.tensor_mul(out_sl, t1, prb)


def _emit(tc, dr):
    nc = tc.nc
    with (
        tc.tile_pool(name="pers", bufs=1) as pers,
        tc.tile_pool(name="ppool", bufs=2, space=bass.MemorySpace.PSUM) as ppool,
    ):
        # ---------------- persistent tiles ----------------
        kt_t = pers.tile([128, 2, N], ATT_DT, tag="kt")     # kn (post norm+rope)
        qt_t = pers.tile([128, 8, R], ATT_DT, tag="qt")     # qn
        vt_t = pers.tile([128, 16, 4 * 65], ES_DT, tag="vt")  # v + ones cols
        ot_t = pers.tile([128, 8, R], PROJ_DT, tag="ot")    # attn out (T layout)
        p2_t = pers.tile([128, 128], F32, tag="p2")
        bcm_t = pers.tile([2, 128], F32, tag="bcm")
        smk_t = pers.tile([128, 2], F32, tag="smk")
        one_t = pers.tile([1, 128], F32, tag="one")
        bq_t = pers.tile([128, 8], F32, tag="bq")
        bk_t = pers.tile([128, 2], F32, tag="bk")
        bv_t = pers.tile([1, 256], F32, tag="bv")
        bp_t = pers.tile([1, 2, 512], F32, tag="bp")
        eps_t = pers.tile([16, 1], F32, tag="eps")

        nc.sync.dma_start(out=p2_t, in_=dr["p2"])
        nc.sync.dma_start(out=bcm_t, in_=dr["bcmask"])
        nc.sync.dma_start(out=smk_t, in_=dr["summask"])
        nc.sync.dma_start(out=one_t, in_=dr["ones1"])
        nc.sync.dma_start(out=bq_t, in_=dr["bq"])
        nc.sync.dma_start(out=bk_t, in_=dr["bk"])
        nc.sync.dma_start(out=bv_t, in_=dr["bv"])
        nc.sync.dma_start(out=bp_t, in_=dr["bp"])
        nc.vector.memset(eps_t, EPS)

        # ================= stage 1: projections + norm + rope =================
        with (
            tc.tile_pool(name="s1", bufs=1) as s1,
            tc.tile_pool(name="s1w", bufs=4) as s1w,
            tc.tile_pool(name="s1t", bufs=2) as s1t,
            tc.tile_pool(name="s1p", bufs=2, space=bass.MemorySpace.PSUM) as s1p,
        ):
            xk_t = s1.tile([128, 8, N], QKV_MM_DT, tag="xk")
            wk_t = s1.tile([128, 8, 256], QKV_MM_DT, tag="wk")
            wv_t = s1.tile([128, 8, 256], QKV_MM_DT, tag="wv")
            ck_t = s1.tile([128, N], F32, tag="ck")
            sk_t = s1.tile([128, N], F32, tag="sk")
            cq_t = s1.tile([128, R], F32, tag="cq")
            sq_t = s1.tile([128, R], F32, tag="sq")
            kraw = s1.tile([128, 2, N], F32, tag="kraw")
            qraw = s1.tile([128, 8, R], F32, tag="qraw")

            for e in range(8):
                nc.sync.dma_start(out=xk_t[:, e, :],
                                  in_=dr["xfT"][128 * e:128 * (e + 1), :])
            for e in range(8):
                nc.sync.dma_start(out=wk_t[:, e, :],
                                  in_=dr["wkT"][128 * e:128 * (e + 1), :])
                nc.sync.dma_start(out=wv_t[:, e, :],
                                  in_=dr["wvT"][128 * e:128 * (e + 1), :])
            nc.sync.dma_start(out=ck_t, in_=dr["ckT"])
            nc.sync.dma_start(out=sk_t, in_=dr["skT"])
            nc.sync.dma_start(out=cq_t, in_=dr["cqT"])
            nc.sync.dma_start(out=sq_t, in_=dr["sqT"])

            # ---- K projection: kraw[:, kt, :] = wkT.T @ xfT + bias ----
            for kt in range(2):
                for nb in range(4):
                    pk = ppool.tile([128, 512], F32, tag="pp", name="pk")
                    for e in range(8):
                        nc.tensor.matmul(pk, wk_t[:, e, 128 * kt:128 * (kt + 1)],
                                         xk_t[:, e, 512 * nb:512 * (nb + 1)],
                                         start=(e == 0), stop=(e == 7))
                    nc.vector.tensor_scalar_add(
                        out=kraw[:, kt, 512 * nb:512 * (nb + 1)], in0=pk,
                        scalar1=bk_t[:, kt:kt + 1])

            # ---- V projection: v[n, dv] + ones cols ----
            for nc16 in range(16):
                pv = ppool.tile([128, 512], F32, tag="pp", name="pv")
                for e in range(8):
                    nc.tensor.matmul(pv[:, 0:256],
                                     xk_t[:, e, 128 * nc16:128 * (nc16 + 1)],
                                     wv_t[:, e, :], start=(e == 0), stop=False)
                nc.tensor.matmul(pv[:, 0:256], one_t, bv_t, start=False, stop=True)
                nc.vector.memset(vt_t[:, nc16, :], 1.0)
                nc.vector.tensor_copy(
                    out=vt_t[:, nc16, :].rearrange("p (g x) -> p g x", g=4)[:, :, 0:64],
                    in_=pv[:, 0:256].rearrange("p (g x) -> p g x", g=4))

            # ---- K rmsnorm + rope (per 512-col block) ----
            for kt in range(2):
                for nb in range(4):
                    sl = slice(512 * nb, 512 * (nb + 1))
                    _norm_rope_block(nc, s1t, s1p,
                                     kraw[:, kt, sl], kt_t[:, kt, sl],
                                     ck_t[:, sl], sk_t[:, sl],
                                     smk_t, bcm_t, p2_t, eps_t)

            # ---- Q projection (weights streamed) + rmsnorm + rope ----
            for qt in range(8):
                pq = ppool.tile([128, 512], F32, tag="pp", name="pq")
                for e in range(8):
                    wq_c = s1w.tile([128, 128], QKV_MM_DT, tag="wqc", name="wqc")
                    nc.sync.dma_start(out=wq_c,
                                      in_=dr["wqT"][128 * e:128 * (e + 1),
                                                    128 * qt:128 * (qt + 1)])
                    nc.tensor.matmul(pq, wq_c, xk_t[:, e, 0:R],
                                     start=(e == 0), stop=(e == 7))
                nc.vector.tensor_scalar_add(out=qraw[:, qt, :], in0=pq,
                                            scalar1=bq_t[:, qt:qt + 1])
                _norm_rope_block(nc, s1t, s1p,
                                 qraw[:, qt, :], qt_t[:, qt, :],
                                 cq_t, sq_t,
                                 smk_t, bcm_t, p2_t, eps_t)

        # ================= stage 2: attention =================
        with (
            tc.tile_pool(name="s2", bufs=2) as s2,
            tc.tile_pool(name="s2s", bufs=3) as s2s,
            tc.tile_pool(name="spool", bufs=3, space=bass.MemorySpace.PSUM) as spool,
        ):
            for t in range(8):
                for r01 in range(2):
                    h = HEAD_ORDER[2 * t + r01]
                    g = h // 4
                    ktile, prow = g // 2, 64 * (g % 2)
                    assert prow == 64 * r01
                    qn_h = qt_t[prow:prow + 64, t, :]
                    et = s2.tile([128, 8, 1024], ES_DT, tag="et", name="et")
                    for w in range(8):
                        ps = spool.tile([128, 1024], F32, tag="sc", name="ps")
                        for c in range(2):
                            nch = 2 * w + c
                            nc.tensor.matmul(
                                ps[:, 512 * c:512 * (c + 1)],
                                kt_t[prow:prow + 64, ktile,
                                     128 * nch:128 * (nch + 1)],
                                qn_h, start=True, stop=True)
                        nc.scalar.activation(out=et[:, w, :], in_=ps, func=AF.Exp,
                                             scale=0.125)
                    po = ppool.tile([128, 512], F32, tag="pp", name="po")
                    for nch in range(16):
                        nc.tensor.matmul(
                            po[0:65, :], vt_t[:, nch, 65 * g:65 * (g + 1)],
                            et[:, nch // 2, 512 * (nch % 2):512 * (nch % 2 + 1)],
                            start=(nch == 0), stop=(nch == 15))
                    rec = s2s.tile([1, 512], F32, tag="rec", name="rec")
                    nc.vector.reciprocal(out=rec, in_=po[64:65, :])
                    prb = ppool.tile([128, 512], F32, tag="pp", name="prb2")
                    nc.tensor.matmul(prb[0:64, :], one_t[:, 0:64], rec,
                                     start=True, stop=True)
                    rb = s2s.tile([64, 512], F32, tag="rb", name="rb")
                    nc.vector.tensor_copy(out=rb, in_=prb[0:64, :])
                    nc.vector.tensor_mul(ot_t[64 * r01:64 * r01 + 64, t, :],
                                         po[0:64, :], rb)

        # ================= stage 3: output projection =================
        with (
            tc.tile_pool(name="s3", bufs=3) as s3,
            tc.tile_pool(name="s3o", bufs=4) as s3o,
            tc.tile_pool(name="fpool", bufs=4, space=bass.MemorySpace.PSUM) as fpool,
        ):
            for half in range(2):
                pf = [fpool.tile([128, 512], F32, tag="f", name=f"pf{half}_{rc}")
                      for rc in range(4)]
                for mt in range(8):
                    pj_c = s3.tile([128, 512], PROJ_DT, tag="pjc", name="pjc")
                    nc.sync.dma_start(out=pj_c,
                                      in_=dr["pjT"][128 * mt:128 * (mt + 1),
                                                    512 * half:512 * (half + 1)])
                    for rc in range(4):
                        nc.tensor.matmul(pf[rc],
                                         ot_t[:, mt, 128 * rc:128 * (rc + 1)],
                                         pj_c, start=(mt == 0), stop=False)
                for rc in range(4):
                    nc.tensor.matmul(pf[rc], one_t, bp_t[:, half, :],
                                     start=False, stop=True)
                    fo = s3o.tile([128, 512], F32, tag="fo", name="fo")
                    nc.vector.tensor_copy(out=fo, in_=pf[rc])
                    nc.sync.dma_start(
                        out=dr["out"][128 * rc:128 * (rc + 1),
                                      512 * half:512 * (half + 1)],
                        in_=fo)


_CACHE = {}


def _get_nc():
    if "nc" in _CACHE:
        return _CACHE["nc"]
    nc = bacc.Bacc("TRN2", target_bir_lowering=False, debug=False,
                   enable_asserts=False, num_devices=8)
    shapes = {
        "xfT": (E, N), "wqT": (E, E), "wkT": (E, 256), "wvT": (E, 256),
        "pjT": (E, E), "ckT": (128, N), "skT": (128, N),
        "cqT": (128, R), "sqT": (128, R), "p2": (128, 128),
        "bcmask": (2, 128), "summask": (128, 2), "ones1": (1, 128),
        "bq": (128, 8), "bk": (128, 2), "bv": (1, 256), "bp": (1, 2, 512),
    }
    bf16_names = {"xfT", "wqT", "wkT", "wvT", "pjT"}
    dr = {k: nc.dram_tensor(k, list(v), BF16 if k in bf16_names else F32,
                            kind="ExternalInput").ap()
          for k, v in shapes.items()}
    dr["out"] = nc.dram_tensor("out", [R, E], F32, kind="ExternalOutput").ap()
    with tile.TileContext(nc) as tc:
        _emit(tc, dr)
    nc.compile()
    _CACHE["nc"] = nc
    return nc


def _host_prep(inputs):
    f = np.float32
    x = np.asarray(inputs["x"], f)
    sin = np.asarray(inputs["sin"], f)
    cos = np.asarray(inputs["cos"], f)
    qn_w = np.asarray(inputs["qn_w"], f)
    kn_w = np.asarray(inputs["kn_w"], f)
    d = np.arange(D)
    sw = d ^ 32
    sign = np.where(d < 32, -1.0, 1.0).astype(f)
    # [64, N] rows indexed by d
    cq64 = (cos * qn_w).T.astype(f)
    sq64 = (sin.T * (sign * qn_w[sw])[:, None]).astype(f)
    ck64 = (cos * kn_w).T.astype(f)
    sk64 = (sin.T * (sign * kn_w[sw])[:, None]).astype(f)
    cq128 = np.tile(cq64, (2, 1))
    sq128 = np.tile(sq64, (2, 1))
    ck128 = np.tile(ck64, (2, 1))
    sk128 = np.tile(sk64, (2, 1))
    p2 = np.zeros((128, 128), f)
    i = np.arange(128)
    p2[i, (i // 64) * 64 + ((i % 64) ^ 32)] = 1.0
    bcm = np.zeros((2, 128), f)
    bcm[0, 0:64] = 1.0
    bcm[1, 64:128] = 1.0
    smk = np.ascontiguousarray(bcm.T)
    ones1 = np.ones((1, 128), f)
    # head permutation: new m index -> old m index
    perm = np.concatenate([np.arange(64 * h, 64 * h + 64) for h in HEAD_ORDER])
    wqT = np.asarray(inputs["wq_w"], f).T   # [e, m]
    pjT = np.asarray(inputs["proj_w"], f).T  # [m, mo]
    bq = np.asarray(inputs["wq_b"], f)
    import ml_dtypes
    bf = ml_dtypes.bfloat16
    com = {
        "wqT": np.ascontiguousarray(wqT[:, perm]).astype(bf),
        "wkT": np.ascontiguousarray(np.asarray(inputs["wk_w"], f).T).astype(bf),
        "wvT": np.ascontiguousarray(np.asarray(inputs["wv_w"], f).T).astype(bf),
        "pjT": np.ascontiguousarray(pjT[perm, :]).astype(bf),
        "p2": p2, "bcmask": bcm, "summask": smk, "ones1": ones1,
        "bq": np.ascontiguousarray(bq[perm].reshape(8, 128).T),
        "bk": np.ascontiguousarray(np.asarray(inputs["wk_b"], f).reshape(2, 128).T),
        "bv": np.asarray(inputs["wv_b"], f).reshape(1, 256),
        "bp": np.asarray(inputs["proj_b"], f).reshape(1, 2, 512),
    }
    in_maps = []
    for c in range(8):
        b, ch = c // 4, c % 4
        roff = R * ch
        m = dict(com)
        m["xfT"] = np.ascontiguousarray(np.roll(x[b].T, -roff, axis=1)).astype(bf)
        m["ckT"] = np.ascontiguousarray(np.roll(ck128, -roff, axis=1))
        m["skT"] = np.ascontiguousarray(np.roll(sk128, -roff, axis=1))
        m["cqT"] = np.ascontiguousarray(cq128[:, roff:roff + R])
        m["sqT"] = np.ascontiguousarray(sq128[:, roff:roff + R])
        in_maps.append(m)
    return in_maps


def kernel(**inputs):
    nc = _get_nc()
    in_maps = _host_prep(inputs)
    res = bass_utils.run_bass_kernel_spmd(nc, in_maps, core_ids=list(range(8)))
    out = np.empty((B, N, E), np.float32)
    for c in range(8):
        b, ch = c // 4, c % 4
        out[b, R * ch:R * (ch + 1), :] = res.results[c]["out"]
    return out

